# revision 34
# baseline (speedup 1.0000x reference)
"""Trainium2 Bass kernel for nn_Dictionary (vq_codebook): out = inp @ Q.T, Q from QR(weight+1e-8).

Strategy (per sharding_hint): data-parallel over batch B=131072 across 8 cores
(16384 rows each); Q.T replicated on every core (QR is tiny, computed on host).

Default mode "f16t": the host transposes inp so the contraction dim i lands on
SBUF partitions with plain contiguous DMAs, and converts operands to fp16
(1 cyc/row on the PE, fp32 PSUM accumulation -> rel L2 err ~3.6e-4 vs the fp32
reference). On each core: stationary = 128x128 blocks of Q.T, moving = [128i,
512b] slices of inpT supertiles, PSUM tiles hold out.T [128j, 512b], accumulated
over the 4 i-tiles; DVE/ACT cast-copy PSUM into wide [128, 4096] fp16 out.T
supertiles so output DMA packets stay >= 2 KB/partition; the host transposes
out.T back and upcasts to fp32. fp16 output halves the output HBM traffic;
per-core traffic is ~32.5 MB and the kernel runs at the PE stream floor
(~131 us/core HW time; 512 matmuls x 216 ns + ramp + drain).
"""

import os

import numpy as np

import concourse.bacc as bacc
import concourse.mybir as mybir
import concourse.tile as tile
from concourse.bass_utils import run_bass_kernel_spmd

N_CORES = 8
B = 131072
D = 512  # contraction dim i (NUM_BASIS)
J = 512  # output dim j (MOTION_DIM)
BC = B // N_CORES  # rows per core
P = 128
KT = D // P  # 4 i-tiles

MODE = os.environ.get("KERNEL_MODE", "f16t")  # f16t | f16 | bf16 | f32r | f16x3

_DT_IN = {
    "f16": mybir.dt.float16,
    "f16t": mybir.dt.float16,
    "bf16": mybir.dt.bfloat16,
    "f32r": mybir.dt.float32r,
    "f16x3": mybir.dt.float16,
}

_compiled = {}
LAST_RESULTS = None  # BassKernelResults of the most recent run (for test.py)


def _np_in_dtype(mode):
    if mode in ("f16", "f16t", "f16x3"):
        return np.float16
    if mode == "bf16":
        import ml_dtypes

        return ml_dtypes.bfloat16
    return np.float32


def _build(mode, bc=BC, chunk=4096, ob=4):
    dt_in = _DT_IN[mode]
    hilo = mode.endswith("x3")
    nc = bacc.Bacc()
    if hilo:
        inpT_hi = nc.dram_tensor("inpT_hi", [D, bc], dt_in, kind="ExternalInput")
        inpT_lo = nc.dram_tensor("inpT_lo", [D, bc], dt_in, kind="ExternalInput")
        qT_hi = nc.dram_tensor("qT_hi", [D, J], dt_in, kind="ExternalInput")
        qT_lo = nc.dram_tensor("qT_lo", [D, J], dt_in, kind="ExternalInput")
        in_drams = [inpT_hi, inpT_lo]
        q_drams = [qT_hi, qT_lo]
    else:
        inpT = nc.dram_tensor("inpT", [D, bc], dt_in, kind="ExternalInput")
        qT = nc.dram_tensor("qT", [D, J], dt_in, kind="ExternalInput")
        in_drams = [inpT]
        q_drams = [qT]
    out = nc.dram_tensor("out", [bc, J], mybir.dt.float32, kind="ExternalOutput")

    BCk = bc
    CHUNK = chunk  # b-columns fetched per supertile DMA (1 MB in fp16)
    OB = ob  # b-tiles batched per output DMA instruction

    # Output viewed as [p, ob-groups, j] so one DMA stores OB b-tiles.
    out3 = out.rearrange("(g ob p) j -> g p ob j", p=P, ob=OB)

    with tile.TileContext(nc) as tc:
        with (
            tc.tile_pool(name="qpool", bufs=1) as qpool,
            tc.tile_pool(name="inpool", bufs=2) as inpool,
            tc.tile_pool(name="outpool", bufs=3) as outpool,
            tc.tile_pool(name="psum", bufs=7, space="PSUM") as psum_pool,
        ):
            # Q.T tiles [i=128, j=512], static for the whole kernel.
            qts = []
            for qi, qd in enumerate(q_drams):
                for it in range(KT):
                    qt_t = qpool.tile([P, J], dt_in, tag=f"qt{qi}_{it}")
                    nc.sync.dma_start(out=qt_t[:], in_=qd[it * P : (it + 1) * P, :])
                    qts.append(qt_t)

            ot = None
            for chunk in range(BCk // CHUNK):
                csl = slice(chunk * CHUNK, (chunk + 1) * CHUNK)
                sups = []  # supertiles per (input, i-tile)
                for ii, ind in enumerate(in_drams):
                    for it in range(KT):
                        sup = inpool.tile([P, CHUNK], dt_in, tag=f"sup{ii}_{it}")
                        # input loads ride the ACT HWDGE ring; output the SP ring
                        nc.scalar.dma_start(
                            out=sup[:], in_=ind[it * P : (it + 1) * P, csl]
                        )
                        sups.append(sup)
                for bt in range(CHUNK // P):
                    bsl = slice(bt * P, (bt + 1) * P)
                    ps = psum_pool.tile([P, J], mybir.dt.float32, tag="ps")
                    if hilo:
                        # out = hi@Qhi + hi@Qlo + lo@Qhi  (drop lo@Qlo)
                        passes = [(0, 0), (0, 1), (1, 0)]
                    else:
                        passes = [(0, 0)]
                    n_mm = len(passes) * KT
                    mm = 0
                    for ii, qi in passes:
                        for it in range(KT):
                            nc.tensor.matmul(
                                ps[:],
                                sups[ii * KT + it][:, bsl],
                                qts[qi * KT + it][:],
                                start=(mm == 0),
                                stop=(mm == n_mm - 1),
                            )
                            mm += 1
                    gbt = chunk * (CHUNK // P) + bt  # global b-tile index
                    if gbt % OB == 0:
                        ot = outpool.tile([P, OB, J], mybir.dt.float32, tag="ot")
                    # split PSUM->SBUF copies across DVE and ACT
                    if gbt % 2 == 0:
                        nc.vector.tensor_copy(out=ot[:, gbt % OB, :], in_=ps[:])
                    else:
                        nc.scalar.copy(out=ot[:, gbt % OB, :], in_=ps[:])
                    if gbt % OB == OB - 1:
                        nc.sync.dma_start(out=out3[gbt // OB], in_=ot[:])
    nc.compile()
    return nc


def _build_t(mode, bc=BC, chunk=2048, outw=4096, warmup_mms=0):
    """Transposed-output variant: PSUM holds [j, b] tiles (stationary = Q.T
    128x128 blocks, moving = inpT [i, b] slices), output written as
    outT [J, bc] fp16 with wide per-partition runs, host transposes back.
    Halves output HBM traffic and keeps DMA packets large (>= 4 KB)."""
    dt_in = _DT_IN[mode]
    assert dt_in == mybir.dt.float16
    nc = bacc.Bacc()
    inpT = nc.dram_tensor("inpT", [D, bc], dt_in, kind="ExternalInput")
    qT = nc.dram_tensor("qT", [D, J], dt_in, kind="ExternalInput")
    outT = nc.dram_tensor("outT", [J, bc], mybir.dt.float16, kind="ExternalOutput")

    NB = 512  # moving free dim per matmul (one PSUM bank of fp32)
    JT = J // P  # 4 j-tiles

    # Input chunk schedule: uniform chunks (leading small chunk measured worse).
    plan = []
    rem = bc
    while rem > 0:
        c = min(chunk, rem)
        plan.append(c)
        rem -= c

    # Output group schedule: small groups at both ends (early first store,
    # short final flush), wide in the middle for large DMA packets.
    ow_plan = []
    rem = bc
    if bc >= 4 * outw:
        for c in (1024, 1024, 2048):
            ow_plan.append(c)
            rem -= c
    tail = [1024, 1024, 2048] if bc >= 4 * outw else []
    rem -= sum(tail)
    while rem > 0:
        c = min(outw, rem)
        ow_plan.append(c)
        rem -= c
    ow_plan.extend(reversed(tail))
    assert sum(ow_plan) == bc and all(w % 512 == 0 for w in ow_plan)
    # column index -> (group_idx, offset, width)
    col2grp = {}
    base = 0
    for gi, w in enumerate(ow_plan):
        for off in range(0, w, 512):
            col2grp[base + off] = (gi, off, w)
        base += w
    grp_base = {}
    base = 0
    for gi, w in enumerate(ow_plan):
        grp_base[gi] = base
        base += w

    with tile.TileContext(nc) as tc:
        with (
            tc.tile_pool(name="qpool", bufs=1) as qpool,
            tc.tile_pool(name="inpool", bufs=4) as inpool,
            tc.tile_pool(name="outpool", bufs=2) as outpool,
            tc.tile_pool(name="psum", bufs=8, space="PSUM") as psum_pool,
            tc.tile_pool(name="warm", bufs=1) as warm_pool,
            tc.tile_pool(name="warmps", bufs=1, space="PSUM") as warmps_pool,
        ):
            # Q.T rows for i-tile `it`: [128i, 512j]; stationary blocks are
            # 128-column slices qts[it][:, jt*128:(jt+1)*128].
            qts = []
            for it in range(KT):
                qt_t = qpool.tile([P, J], dt_in, tag=f"qt{it}")
                nc.gpsimd.dma_start(out=qt_t[:], in_=qT[it * P : (it + 1) * P, :])
                qts.append(qt_t)

            if warmup_mms:
                # Warmup matmuls on the (tiny, early-arriving) qT tiles: keeps
                # the PE HAM busy while the first input chunk streams in, so
                # real matmuls start un-throttled. Result bank is never read.
                wps = warmps_pool.tile([P, NB], mybir.dt.float32, tag="wps")
                for wi in range(warmup_mms):
                    nc.tensor.matmul(
                        wps[:],
                        qts[0][:, :P],
                        qts[0][:],
                        start=(wi == 0),
                        stop=(wi == warmup_mms - 1),
                    )

            ots = [None] * JT
            col_base = 0
            for chunk_i, csz in enumerate(plan):
                csl = slice(col_base, col_base + csz)
                sups = []
                for it in range(KT):
                    sup = inpool.tile([P, csz], dt_in, tag=f"sup{it}")
                    nc.scalar.dma_start(
                        out=sup[:], in_=inpT[it * P : (it + 1) * P, csl]
                    )
                    sups.append(sup)
                for bn in range(csz // NB):
                    col0 = col_base + bn * NB
                    gi, goff, gw = col2grp[col0]
                    if goff == 0:
                        for jt in range(JT):
                            ots[jt] = outpool.tile(
                                [P, outw],
                                mybir.dt.float16,
                                tag=f"ot{jt}",
                                name=f"ot{jt}",
                            )
                    osl = slice(goff, goff + NB)
                    bsl = slice(bn * NB, (bn + 1) * NB)
                    for jt in range(JT):
                        ps = psum_pool.tile([P, NB], mybir.dt.float32, tag="ps")
                        for it in range(KT):
                            nc.tensor.matmul(
                                ps[:],
                                qts[it][:, jt * P : (jt + 1) * P],
                                sups[it][:, bsl],
                                start=(it == 0),
                                stop=(it == KT - 1),
                            )
                        if jt % 2 == 0:
                            nc.vector.tensor_copy(out=ots[jt][:, osl], in_=ps[:])
                        else:
                            nc.scalar.copy(out=ots[jt][:, osl], in_=ps[:])
                    if goff + NB == gw:
                        g0 = grp_base[gi]
                        for jt in range(JT):
                            nc.sync.dma_start(
                                out=outT[jt * P : (jt + 1) * P, g0 : g0 + gw],
                                in_=ots[jt][:, :gw],
                            )
                col_base += csz
    nc.compile()
    return nc


def _get_nc(mode):
    if mode not in _compiled:
        if mode == "f16t":
            _compiled[mode] = _build_t(mode)
        else:
            _compiled[mode] = _build(mode)
    return _compiled[mode]


def kernel(inp: np.ndarray, weight: np.ndarray) -> np.ndarray:
    global LAST_RESULTS
    mode = MODE
    nc = _get_nc(mode)

    w = np.asarray(weight, dtype=np.float32) + np.float32(1e-8)
    Q = np.linalg.qr(w)[0].astype(np.float32)  # [J, D] == [512, 512]
    np_dt = _np_in_dtype(mode)

    inp = np.asarray(inp, dtype=np.float32)
    inpT = inp.T  # [D, B] view

    QT = Q.T  # QT[i, j] = Q[j, i]
    in_maps = []
    if mode.endswith("x3"):
        qt_hi = QT.astype(np_dt)
        qt_lo = (QT - qt_hi.astype(np.float32)).astype(np_dt)
        for c in range(N_CORES):
            sl = inpT[:, c * BC : (c + 1) * BC].astype(np.float32)
            hi = sl.astype(np_dt)
            lo = (sl - hi.astype(np.float32)).astype(np_dt)
            in_maps.append(
                {"inpT_hi": hi, "inpT_lo": lo, "qT_hi": qt_hi, "qT_lo": qt_lo}
            )
    else:
        qt16 = np.ascontiguousarray(QT).astype(np_dt)
        for c in range(N_CORES):
            in_maps.append(
                {"inpT": inpT[:, c * BC : (c + 1) * BC].astype(np_dt), "qT": qt16}
            )

    # First execution of a freshly compiled NEFF occasionally dies with
    # NRT_EXEC_UNIT_UNRECOVERABLE (transient, esp. with profiling on);
    # a straight retry has always succeeded.
    last_exc = None
    for _attempt in range(3):
        try:
            res = run_bass_kernel_spmd(nc, in_maps, list(range(N_CORES)))
            break
        except Exception as e:  # noqa: BLE001
            last_exc = e
            import time as _time

            _time.sleep(2.0)
    else:
        raise last_exc
    LAST_RESULTS = res
    if mode == "f16t":
        out = np.empty((B, J), dtype=np.float32)
        for c in range(N_CORES):
            # outT [J, BC] fp16 -> out rows [c*BC:(c+1)*BC] fp32
            out[c * BC : (c + 1) * BC, :] = res.results[c]["outT"].T
        return out
    return np.concatenate([res.results[c]["out"] for c in range(N_CORES)], axis=0)
